# revision 7
# baseline (speedup 1.0000x reference)
"""Fused MoE (top-2 routing) on 8 trn2 NeuronCores, expert-parallel.

Strategy: E=16 experts are sharded 2-per-core. The host groups the T*TOPK
(token, slot) pairs by expert (the all-to-all "dispatch"), pads each expert's
token list to a fixed capacity, and ships each core:
  - xT  [K, 2*CAP]     gathered tokens, transposed (k on rows)
  - wup [2*K, 2H]      up_weight[e].T for its 2 experts (k-major)
  - wdn [2*H, K]       down_weight[e].T for its 2 experts (h-major)
  - wv  [2*CAP, 1]     routing weight per gathered pair
The device computes, per expert:  up.T = wupT.T-chunks @ xT  (PSUM, fp32r),
SwiGLU in the transposed layout (no on-chip transposes needed anywhere),
down = actT.T @ wdnT with the per-pair routing weight applied during the
PSUM->SBUF copy, and writes y [2*CAP, K]. The host scatter-adds y rows back
to tokens (the "combine").
"""

import numpy as np

import concourse.bass as bass
import concourse.mybir as mybir
from concourse.bass_utils import run_bass_kernel_spmd
from concourse.tile import TileContext

T, K, H, E, TOPK = 4096, 1024, 256, 16, 2
H2 = 2 * H  # 512
NCORES = 8
EPC = E // NCORES  # experts per core = 2
CAP = 640  # token-pair capacity per expert (max observed ~550 of mean 512)
PAIRS = EPC * CAP  # 1280 rows per core
UPCHUNK = 320  # up-GEMM token tile (>=256 keeps fp32r at full rate)
F32 = mybir.dt.float32
F32R = mybir.dt.float32r


def _fix_multi_waits(nc):
    """This walrus build accepts one sync-wait command per instruction (two
    for EventSemaphore); Tile's exit drain stacks every outstanding semaphore
    onto a single Drain. Move the excess waits onto no-ops inserted before
    the offending instruction on the same engine."""
    for f in nc.m.functions:
        for bb in f.blocks:
            i = 0
            while i < len(bb.instructions):
                ins = bb.instructions[i]
                si = ins.sync_info
                cap = 2 if isinstance(ins, mybir.InstEventSemaphore) else 1
                if si is not None and si.on_wait and len(si.on_wait) > cap:
                    waits = list(si.on_wait)
                    keep, extra = waits[:cap], waits[cap:]
                    nops = [
                        mybir.InstNoOp(
                            name=f"{ins.name}_waitfix{j}",
                            sync_info=mybir.SyncInfo(on_wait=[w], on_update=[]),
                            bass_nofuse=True,
                            engine=ins.engine,
                        )
                        for j, w in enumerate(extra)
                    ]
                    ins.sync_info = mybir.SyncInfo(
                        on_wait=keep, on_update=list(si.on_update)
                    )
                    bb.instructions[i:i] = nops
                    i += len(nops)
                i += 1


_NC = None


def _build():
    global _NC
    if _NC is not None:
        return _NC
    nc = bass.Bass()
    xT = nc.dram_tensor("xT", [K, PAIRS], F32R, kind="ExternalInput")
    wup = nc.dram_tensor("wup", [EPC * K, H2], F32R, kind="ExternalInput")
    wdn = nc.dram_tensor("wdn", [EPC * H, K], F32R, kind="ExternalInput")
    wv = nc.dram_tensor("wv", [PAIRS, 1], F32, kind="ExternalInput")
    y = nc.dram_tensor("y", [PAIRS, K], F32, kind="ExternalOutput")

    KC = K // 128  # 8 contraction chunks
    NT = CAP // UPCHUNK  # 2 up token-tiles per expert
    ND = CAP // 128  # 5 down token-tiles per expert

    with TileContext(nc) as tc:
        with (
            tc.tile_pool(name="persist", bufs=1) as pp,
            tc.tile_pool(name="sil", bufs=4) as silp,
            tc.tile_pool(name="yout", bufs=3) as yp,
            tc.tile_pool(name="psum_up", bufs=1, space="PSUM") as psu,
            tc.tile_pool(name="psum_dn", bufs=2, space="PSUM") as psd,
        ):
            xsb = pp.tile([128, KC, PAIRS], F32R)
            wupsb = pp.tile([128, EPC, KC, H2], F32R)
            wdnsb = pp.tile([128, EPC, 2, K], F32R)
            actsb = pp.tile([128, EPC, 2, CAP], F32R)
            wvsb = pp.tile([128, PAIRS // 128], F32)

            # loads (split so they spread across DMA queues)
            for kc in range(KC):
                nc.sync.dma_start(
                    xsb[:, kc, :], xT[kc * 128 : (kc + 1) * 128, :]
                )
            for el in range(EPC):
                for kg in range(2):
                    src = wup[el * K + kg * 512 : el * K + (kg + 1) * 512, :]
                    nc.sync.dma_start(
                        wupsb[:, el, kg * 4 : (kg + 1) * 4, :],
                        src.rearrange("(kc p) f -> p kc f", p=128),
                    )
                for hh in range(2):
                    nc.sync.dma_start(
                        wdnsb[:, el, hh, :],
                        wdn[el * H + hh * 128 : el * H + (hh + 1) * 128, :],
                    )
            nc.sync.dma_start(wvsb[:], wv.rearrange("(c p) o -> p (c o)", p=128))

            # up GEMM + SwiGLU, all in the [feature-on-partition, token-free]
            # layout; gate = features 0:256, proj = 256:512
            for el in range(EPC):
                for ti in range(NT):
                    c0 = el * CAP + ti * UPCHUNK
                    pts = [
                        psu.tile([128, 512], F32, tag=f"up{ff}", name=f"up{ff}")[
                            :, :UPCHUNK
                        ]
                        for ff in range(4)
                    ]
                    for kc in range(KC):
                        rhs = xsb[:, kc, c0 : c0 + UPCHUNK]
                        for ff in range(4):
                            nc.tensor.matmul(
                                pts[ff],
                                wupsb[:, el, kc, ff * 128 : (ff + 1) * 128],
                                rhs,
                                start=(kc == 0),
                                stop=(kc == KC - 1),
                            )
                    for hh in range(2):
                        sil = silp.tile([128, UPCHUNK], F32, tag="sil")
                        nc.scalar.activation(
                            sil[:], pts[hh], mybir.ActivationFunctionType.Silu
                        )
                        nc.vector.tensor_tensor(
                            actsb[
                                :, el, hh, ti * UPCHUNK : (ti + 1) * UPCHUNK
                            ],
                            sil[:],
                            pts[2 + hh],
                            mybir.AluOpType.mult,
                        )

            # down GEMM [token-on-partition, k-free], weight applied on the
            # PSUM->SBUF copy, then store
            for el in range(EPC):
                for td in range(ND):
                    pys = [
                        psd.tile([128, 512], F32, tag=f"dn{nn}", name=f"dn{nn}")
                        for nn in range(2)
                    ]
                    for nn in range(2):
                        for hh in range(2):
                            nc.tensor.matmul(
                                pys[nn][:],
                                actsb[:, el, hh, td * 128 : (td + 1) * 128],
                                wdnsb[:, el, hh, nn * 512 : (nn + 1) * 512],
                                start=(hh == 0),
                                stop=(hh == 1),
                            )
                    ysb = yp.tile([128, K], F32, tag="y")
                    col = el * ND + td
                    for nn in range(2):
                        nc.vector.tensor_scalar_mul(
                            ysb[:, nn * 512 : (nn + 1) * 512],
                            pys[nn][:],
                            wvsb[:, col : col + 1],
                        )
                    r0 = el * CAP + td * 128
                    nc.sync.dma_start(y[r0 : r0 + 128, :], ysb[:])

    _fix_multi_waits(nc)
    _NC = nc
    return nc


last_results = None  # BassKernelResults of the most recent launch (for test.py)


def kernel(hidden_states, topk_weights, topk_ids, up_weight, down_weight):
    global last_results
    hs = np.ascontiguousarray(np.asarray(hidden_states, dtype=np.float32))
    twf = np.asarray(topk_weights, dtype=np.float32).ravel()
    ids = np.asarray(topk_ids).astype(np.int64).ravel()
    wu = np.asarray(up_weight, dtype=np.float32)
    wd = np.asarray(down_weight, dtype=np.float32)

    nc = _build()

    order = np.argsort(ids, kind="stable")
    counts = np.bincount(ids, minlength=E)
    starts = np.concatenate([[0], np.cumsum(counts)])
    hsT = np.ascontiguousarray(hs.T)  # [K, T]

    wup_maps = []
    wdn_maps = []
    for c in range(NCORES):
        es = range(EPC * c, EPC * (c + 1))
        wup_maps.append(
            np.ascontiguousarray(
                np.concatenate([wu[e].T for e in es], axis=0)
            )
        )
        wdn_maps.append(
            np.ascontiguousarray(
                np.concatenate([wd[e].T for e in es], axis=0)
            )
        )

    out = np.zeros((T, K), np.float32)
    rounds = int(max(1, -(-int(counts.max()) // CAP)))
    for r in range(rounds):
        in_maps = []
        toks = []  # per core: list of (el, n, token_idx)
        for c in range(NCORES):
            xTa = np.zeros((K, PAIRS), np.float32)
            wva = np.zeros((PAIRS, 1), np.float32)
            ct = []
            for el in range(EPC):
                e = EPC * c + el
                lo = starts[e] + r * CAP
                hi = min(starts[e + 1], lo + CAP)
                seg = order[lo:hi] if hi > lo else np.empty(0, np.int64)
                n = len(seg)
                if n:
                    t = seg // TOPK
                    xTa[:, el * CAP : el * CAP + n] = hsT[:, t]
                    wva[el * CAP : el * CAP + n, 0] = twf[seg]
                    ct.append((el, n, t))
            toks.append(ct)
            in_maps.append(
                {"xT": xTa, "wup": wup_maps[c], "wdn": wdn_maps[c], "wv": wva}
            )
        last_results = run_bass_kernel_spmd(
            nc, in_maps, core_ids=list(range(NCORES))
        )
        for c in range(NCORES):
            yc = last_results.results[c]["y"]
            for el, n, t in toks[c]:
                np.add.at(out, t, yc[el * CAP : el * CAP + n])
    return out


# revision 8
# speedup vs baseline: 1.1107x; 1.1107x over previous
"""Fused MoE (top-2 routing) on 8 trn2 NeuronCores, expert-parallel.

Strategy: E=16 experts are sharded 2-per-core. The host groups the T*TOPK
(token, slot) pairs by expert (the all-to-all "dispatch"), pads each expert's
token list to a fixed capacity CAP, and ships each core:
  - xT  [K, 2*CAP]     gathered tokens, transposed (k on rows)
  - wup [2*K, 2H]      up_weight[e].T for its 2 experts (k-major)
  - wdn [2*H, K]       down_weight[e].T for its 2 experts (h-major)
  - wv  [2*CAP, 1]     routing weight per gathered pair
The device computes, per expert:  up.T = wupT-chunks.T @ xT  (PSUM, fp32
accumulate), SwiGLU in the transposed layout (no on-chip transposes needed
anywhere), down = actT.T @ wdnT with the per-pair routing weight applied on
the PSUM->SBUF copy, and writes y [2*CAP, K]. The host scatter-adds y rows
back to tokens (the "combine").

Tiles are split per (expert, k-chunk) and loads are ordered expert-0-first so
the up GEMM starts as soon as the first chunks land instead of after the full
load phase.
"""

import os

import ml_dtypes
import numpy as np

import concourse.bass as bass
import concourse.mybir as mybir
from concourse.bass_utils import run_bass_kernel_spmd
from concourse.tile import TileContext

T, K, H, E, TOPK = 4096, 1024, 256, 16, 2
H2 = 2 * H  # 512
NCORES = 8
EPC = E // NCORES  # experts per core = 2
CAP = 640  # token-pair capacity per expert (max observed ~550 of mean 512)
PAIRS = EPC * CAP  # 1280 rows per core
UPCHUNK = 320  # up-GEMM token tile (>=256 keeps fp32r at full rate)
KC = K // 128  # 8 contraction chunks
NT = CAP // UPCHUNK  # up token-tiles per expert
ND = CAP // 128  # down token-tiles per expert

F32 = mybir.dt.float32
# matmul input dtype: "f32r" (tf32-like, full DMA bytes) or "bf16" (half DMA)
MM_DTYPE = os.environ.get("MOE_MM_DTYPE", "f32r")


def _fix_multi_waits(nc):
    """This walrus build accepts one sync-wait command per instruction (two
    for EventSemaphore); Tile's exit drain stacks every outstanding semaphore
    onto a single Drain. Move the excess waits onto no-ops inserted before
    the offending instruction on the same engine."""
    for f in nc.m.functions:
        for bb in f.blocks:
            i = 0
            while i < len(bb.instructions):
                ins = bb.instructions[i]
                si = ins.sync_info
                cap = 2 if isinstance(ins, mybir.InstEventSemaphore) else 1
                if si is not None and si.on_wait and len(si.on_wait) > cap:
                    waits = list(si.on_wait)
                    keep, extra = waits[:cap], waits[cap:]
                    nops = [
                        mybir.InstNoOp(
                            name=f"{ins.name}_waitfix{j}",
                            sync_info=mybir.SyncInfo(on_wait=[w], on_update=[]),
                            bass_nofuse=True,
                            engine=ins.engine,
                        )
                        for j, w in enumerate(extra)
                    ]
                    ins.sync_info = mybir.SyncInfo(
                        on_wait=keep, on_update=list(si.on_update)
                    )
                    bb.instructions[i:i] = nops
                    i += len(nops)
                i += 1


_NC = None


def _build():
    global _NC
    if _NC is not None:
        return _NC
    DT = mybir.dt.float32r if MM_DTYPE == "f32r" else mybir.dt.bfloat16
    nc = bass.Bass()
    xT = nc.dram_tensor("xT", [K, PAIRS], DT, kind="ExternalInput")
    wup = nc.dram_tensor("wup", [EPC * K, H2], DT, kind="ExternalInput")
    wdn = nc.dram_tensor("wdn", [EPC * H, K], DT, kind="ExternalInput")
    wv = nc.dram_tensor("wv", [PAIRS, 1], F32, kind="ExternalInput")
    y = nc.dram_tensor("y", [PAIRS, K], F32, kind="ExternalOutput")

    with TileContext(nc) as tc:
        with (
            tc.tile_pool(name="persist", bufs=1) as pp,
            tc.tile_pool(name="sil", bufs=4) as silp,
            tc.tile_pool(name="yout", bufs=3) as yp,
            tc.tile_pool(name="psum_up", bufs=1, space="PSUM") as psu,
            tc.tile_pool(name="psum_dn", bufs=2, space="PSUM") as psd,
        ):
            # one tile per (tensor, expert, chunk) so readers only gate on
            # the DMA that actually feeds them
            xsb = [
                [
                    pp.tile([128, CAP], DT, tag=f"x{el}_{kc}", name=f"x{el}_{kc}")
                    for kc in range(KC)
                ]
                for el in range(EPC)
            ]
            wupsb = [
                [
                    pp.tile(
                        [128, 4, H2], DT, tag=f"wu{el}_{kg}", name=f"wu{el}_{kg}"
                    )
                    for kg in range(2)
                ]
                for el in range(EPC)
            ]
            wdnsb = [
                [
                    pp.tile([128, K], DT, tag=f"wd{el}_{hh}", name=f"wd{el}_{hh}")
                    for hh in range(2)
                ]
                for el in range(EPC)
            ]
            actsb = [
                [
                    pp.tile([128, CAP], DT, tag=f"a{el}_{hh}", name=f"a{el}_{hh}")
                    for hh in range(2)
                ]
                for el in range(EPC)
            ]
            wvsb = pp.tile([128, PAIRS // 128], F32)

            # loads, expert-0's working set first so its GEMMs start early
            for el in range(EPC):
                for kg in range(2):
                    nc.sync.dma_start(
                        wupsb[el][kg][:],
                        wup[
                            el * K + kg * 512 : el * K + (kg + 1) * 512, :
                        ].rearrange("(kc p) f -> p kc f", p=128),
                    )
                for hh in range(2):
                    nc.sync.dma_start(
                        wdnsb[el][hh][:],
                        wdn[el * H + hh * 128 : el * H + (hh + 1) * 128, :],
                    )
                if el == 0:
                    nc.sync.dma_start(
                        wvsb[:], wv.rearrange("(c p) o -> p (c o)", p=128)
                    )
                for kc in range(KC):
                    nc.sync.dma_start(
                        xsb[el][kc][:],
                        xT[
                            kc * 128 : (kc + 1) * 128,
                            el * CAP : (el + 1) * CAP,
                        ],
                    )

            def up_phase(el):
                # up.T in PSUM: [feature-on-partition, token-free];
                # gate = features 0:256, proj = 256:512
                for ti in range(NT):
                    c0 = ti * UPCHUNK
                    pts = [
                        psu.tile([128, 512], F32, tag=f"up{ff}", name=f"up{ff}")[
                            :, :UPCHUNK
                        ]
                        for ff in range(4)
                    ]
                    for kc in range(KC):
                        rhs = xsb[el][kc][:, c0 : c0 + UPCHUNK]
                        for ff in range(4):
                            nc.tensor.matmul(
                                pts[ff],
                                wupsb[el][kc // 4][
                                    :, kc % 4, ff * 128 : (ff + 1) * 128
                                ],
                                rhs,
                                start=(kc == 0),
                                stop=(kc == KC - 1),
                            )
                    for hh in range(2):
                        sil = silp.tile([128, UPCHUNK], F32, tag="sil")
                        nc.scalar.activation(
                            sil[:], pts[hh], mybir.ActivationFunctionType.Silu
                        )
                        nc.vector.tensor_tensor(
                            actsb[el][hh][:, c0 : c0 + UPCHUNK],
                            sil[:],
                            pts[2 + hh],
                            mybir.AluOpType.mult,
                        )

            def down_phase(el):
                # down: [token-on-partition, k-free]; routing weight applied
                # on the PSUM->SBUF copy (split across DVE and ACT)
                for td in range(ND):
                    pys = [
                        psd.tile([128, 512], F32, tag=f"dn{nn}", name=f"dn{nn}")
                        for nn in range(2)
                    ]
                    for nn in range(2):
                        for hh in range(2):
                            nc.tensor.matmul(
                                pys[nn][:],
                                actsb[el][hh][:, td * 128 : (td + 1) * 128],
                                wdnsb[el][hh][:, nn * 512 : (nn + 1) * 512],
                                start=(hh == 0),
                                stop=(hh == 1),
                            )
                    ysb = yp.tile([128, K], F32, tag="y")
                    col = el * ND + td
                    wcol = wvsb[:, col : col + 1]
                    nc.vector.tensor_scalar_mul(ysb[:, 0:512], pys[0][:], wcol)
                    nc.scalar.mul(ysb[:, 512:1024], pys[1][:], wcol)
                    r0 = el * CAP + td * 128
                    nc.sync.dma_start(y[r0 : r0 + 128, :], ysb[:])

            up_phase(0)
            up_phase(1)
            down_phase(0)
            down_phase(1)

    _fix_multi_waits(nc)
    _NC = nc
    return nc


last_results = None  # BassKernelResults of the most recent launch (for test.py)


def kernel(hidden_states, topk_weights, topk_ids, up_weight, down_weight):
    global last_results
    np_dt = np.float32 if MM_DTYPE == "f32r" else ml_dtypes.bfloat16
    hs = np.asarray(hidden_states, dtype=np.float32)
    twf = np.asarray(topk_weights, dtype=np.float32).ravel()
    ids = np.asarray(topk_ids).astype(np.int64).ravel()
    wu = np.asarray(up_weight, dtype=np.float32)
    wd = np.asarray(down_weight, dtype=np.float32)

    nc = _build()

    order = np.argsort(ids, kind="stable")
    counts = np.bincount(ids, minlength=E)
    starts = np.concatenate([[0], np.cumsum(counts)])
    hsT = np.ascontiguousarray(hs.T.astype(np_dt))  # [K, T]

    wup_maps = []
    wdn_maps = []
    for c in range(NCORES):
        es = range(EPC * c, EPC * (c + 1))
        wup_maps.append(
            np.ascontiguousarray(
                np.concatenate([wu[e].T for e in es], axis=0).astype(np_dt)
            )
        )
        wdn_maps.append(
            np.ascontiguousarray(
                np.concatenate([wd[e].T for e in es], axis=0).astype(np_dt)
            )
        )

    out = np.zeros((T, K), np.float32)
    rounds = int(max(1, -(-int(counts.max()) // CAP)))
    for r in range(rounds):
        in_maps = []
        toks = []  # per core: list of (el, n, token_idx)
        for c in range(NCORES):
            xTa = np.zeros((K, PAIRS), np_dt)
            wva = np.zeros((PAIRS, 1), np.float32)
            ct = []
            for el in range(EPC):
                e = EPC * c + el
                lo = starts[e] + r * CAP
                hi = min(starts[e + 1], lo + CAP)
                seg = order[lo:hi] if hi > lo else np.empty(0, np.int64)
                n = len(seg)
                if n:
                    t = seg // TOPK
                    xTa[:, el * CAP : el * CAP + n] = hsT[:, t]
                    wva[el * CAP : el * CAP + n, 0] = twf[seg]
                    ct.append((el, n, t))
            toks.append(ct)
            in_maps.append(
                {"xT": xTa, "wup": wup_maps[c], "wdn": wdn_maps[c], "wv": wva}
            )
        last_results = run_bass_kernel_spmd(
            nc, in_maps, core_ids=list(range(NCORES))
        )
        for c in range(NCORES):
            yc = last_results.results[c]["y"]
            for el, n, t in toks[c]:
                np.add.at(out, t, yc[el * CAP : el * CAP + n])
    return out
